# revision 27
# baseline (speedup 1.0000x reference)
"""RNN-T JointNet fused Bass kernel for Trainium2, SPMD over 8 NeuronCores.

Reference computation (all fp32):
    enc = LN(encoder_out @ W_enc + b_enc) * g_enc + be_enc      # [B,T,J]
    dec = LN(decoder_out @ W_dec + b_dec) * g_dec + be_dec      # [B,U,J]
    joint = relu(enc[:,:,None,:] + dec[:,None,:,:])             # [B,T,U,J]
    out = joint @ W_out + b_out                                 # [B,T,U,V]

Shapes: B=4, T=512, U=64, E=D=J=512, V=1024.

Sharding: data-parallel over flattened (B,T): core c owns b = c//2,
t in [(c%2)*256, ...+256) -> 16384 contiguous output rows.

vs the bf16 baseline (253us, PE-bound: 1024 N=512 bf16 matmuls @222ns):
run the big GEMM in fp8e4 DoubleRow (2x contraction/pass). Plain fp8 of
relu(x) and W gives 3.9% rel err (gate 2e-2), so split off the optimal
affine part of the relu and compute it exactly:

    relu(x) = 0.5*x + b + g(x),  g = 0.5*|x| - b,  b = E[0.5|x|]
    out = [0.5*enc@W]_t + [0.5*dec@W + b*colsum(W) + b_out]_u + g@W

E[g^2] ~ 0.18 vs E[relu^2] ~ 1.0 so the fp8 quantization noise drops
~sqrt(5.5)x: measured 1.67e-2 end to end.

The linear terms are folded into PSUM by the PE itself (the elementwise
engines have no capacity at fp8 cadence: DVE ~100G, GpSimd ~46G elem/s,
ACT can only copy/activate): per 128-row block, one extra bf16 matmul
with stationary S66 [66,128] one-hot rows (2 t-selectors + 64
u-selectors, host-supplied constant) and moving L [66,V] = [encW rows
t0,t0+1; decB (incl b*colsum(W)+b_out)]. L tiles are assembled by tiny
SBUF->SBUF DMAs (2 rows per block from encW) two supertiles ahead;
PSUM evictions stay plain copies (ACT 3/4, DVE 1/4).

Supertile (512 rows): 4 row-blocks x [2x2 DR fp8 MMs (N=512) + 2 lin
MMs] ~ 5.8us PE; joint build (DVE 1/4, GpSimd 3/4), g-quant (single
DVE tensor_scalar: abs_max 0 -> subtract 2b -> fp8 out), both
software-pipelined one supertile ahead. Scales: w8 = fp8(W*128), w_lin
= bf16(W*128); device output is 256*out bf16, host divides by 256.
"""

import numpy as np

B, T, U = 4, 512, 64
E = D = J = 512
V = 1024
EPS = 1e-5
P = 128
NCORES = 8
TC = T * B // NCORES            # 256 t-rows per core
ROWS = TC * U                   # 16384 output rows per core
NMM = ROWS // 512               # 32 supertiles of 512 rows (8 t values)
KJ = J // P                     # 4 contraction blocks
TSUP = 512 // U                 # 8 t values per supertile
LBUF = 4                        # L-tile rotation depth
WS = 128.0                      # weight scale baked into w8/w_lin
OS = 1.0 / 256.0                # host-side output descale

_CACHE = {}


def _build(apply_b_enc, apply_g_enc, apply_be_enc,
           apply_b_dec, apply_g_dec, apply_be_dec, apply_b_out):
    import concourse.bass as bass
    import concourse.mybir as mybir
    import concourse.tile as tile
    from concourse import bacc
    from concourse.masks import make_identity

    f32 = mybir.dt.float32
    bf16 = mybir.dt.bfloat16
    f8 = mybir.dt.float8e4
    AF = mybir.ActivationFunctionType
    OP = mybir.AluOpType
    DR = mybir.MatmulPerfMode.DoubleRow

    nc = bacc.Bacc(target_bir_lowering=False)

    enc_xT = nc.dram_tensor("enc_xT", [P, E // P, TC], bf16, kind="ExternalInput")
    dec_xT = nc.dram_tensor("dec_xT", [P, D // P, U], bf16, kind="ExternalInput")
    w_enc = nc.dram_tensor("w_enc", [P, E // P, J], bf16, kind="ExternalInput")
    w_dec = nc.dram_tensor("w_dec", [P, D // P, J], bf16, kind="ExternalInput")
    w8 = nc.dram_tensor("w8", [P, KJ, V], f8, kind="ExternalInput")
    w_lin = nc.dram_tensor("w_lin", [P, KJ, V], bf16, kind="ExternalInput")
    s66 = nc.dram_tensor("s66", [2 + U, P], bf16, kind="ExternalInput")
    c2b = nc.dram_tensor("c2b", [1], f32, kind="ExternalInput")
    b_enc = nc.dram_tensor("b_enc", [J], f32, kind="ExternalInput")
    g_enc = nc.dram_tensor("g_enc", [J], f32, kind="ExternalInput")
    be_enc = nc.dram_tensor("be_enc", [J], f32, kind="ExternalInput")
    b_dec = nc.dram_tensor("b_dec", [J], f32, kind="ExternalInput")
    g_dec = nc.dram_tensor("g_dec", [J], f32, kind="ExternalInput")
    be_dec = nc.dram_tensor("be_dec", [J], f32, kind="ExternalInput")
    b_out = nc.dram_tensor("b_out", [V], f32, kind="ExternalInput")
    out = nc.dram_tensor("out", [ROWS, V], bf16, kind="ExternalOutput")

    def bcast_row(dram_vec, n):
        ap = dram_vec[:]
        return bass.AP(tensor=ap.tensor, offset=ap.offset,
                       ap=[[0, P], [1, n]])

    from contextlib import ExitStack, contextmanager

    @contextmanager
    def _null_ctx():
        yield

    with tile.TileContext(nc) as tc, ExitStack() as ctx:
        const = ctx.enter_context(tc.tile_pool(name="const", bufs=1))
        prep = ctx.enter_context(tc.tile_pool(name="prep", bufs=2))
        jpool = ctx.enter_context(tc.tile_pool(name="jpool", bufs=3))
        jqpool = ctx.enter_context(tc.tile_pool(name="jqpool", bufs=3))
        opool = ctx.enter_context(tc.tile_pool(name="opool", bufs=10))
        mpsum = ctx.enter_context(tc.tile_pool(name="mpsum", bufs=4, space="PSUM"))

        # ---- input DMAs, priority-ordered on the SP queue ----
        xT_sb = prep.tile([P, E // P, TC], bf16, tag="xT_sb")
        nc.sync.dma_start(xT_sb[:], enc_xT[:])
        wenc_sb = const.tile([P, E // P, J], bf16)
        nc.sync.dma_start(wenc_sb[:], w_enc[:])
        dxT_sb = prep.tile([P, D // P, U], bf16, tag="dxT_sb")
        nc.sync.dma_start(dxT_sb[:], dec_xT[:])
        wdec_sb = const.tile([P, D // P, J], bf16)
        nc.sync.dma_start(wdec_sb[:], w_dec[:])
        wlin_sb = const.tile([P, KJ, V], bf16)
        nc.sync.dma_start(wlin_sb[:], w_lin[:])
        w8_sb = const.tile([P, KJ, V], f8)
        nc.sync.dma_start(w8_sb[:], w8[:])

        ident = const.tile([P, P], bf16)
        make_identity(nc, ident)

        warm_src = const.tile([P, 512], bf16)
        nc.vector.memset(warm_src[:], 0.0)
        onesJ = const.tile([P, 1], bf16)
        nc.vector.memset(onesJ[:], 1.0)
        ones_u = const.tile([1, U], bf16)
        nc.vector.memset(ones_u[:], 1.0)

        def warmup(n, label):
            for i in range(n):
                wps = mpsum.tile([P, 512], f32, tag="mps", name=f"w{label}_{i}")
                nc.tensor.matmul(wps, warm_src[:, :P], warm_src[:],
                                 start=True, stop=True)

        eps_sb = const.tile([P, 1], f32)
        nc.vector.memset(eps_sb[:], EPS)
        zero_sb = const.tile([P, 1], f32)
        nc.vector.memset(zero_sb[:], 0.0)

        def load_vec(vec, n, enabled):
            if not enabled:
                return None
            t = const.tile([P, n], f32)
            nc.scalar.dma_start(t[:], bcast_row(vec, n))
            return t

        s66_sb = const.tile([2 + U, P], bf16)
        nc.scalar.dma_start(s66_sb[:], s66[:])
        c2b_sb = const.tile([P, 1], f32)
        nc.scalar.dma_start(c2b_sb[:], bcast_row(c2b, 1))
        b_enc_sb = load_vec(b_enc, J, apply_b_enc)
        g_enc_sb = load_vec(g_enc, J, apply_g_enc)
        be_enc_sb = load_vec(be_enc, J, apply_be_enc)
        b_dec_sb = load_vec(b_dec, J, apply_b_dec)
        g_dec_sb = load_vec(g_dec, J, apply_g_dec)
        be_dec_sb = load_vec(be_dec, J, apply_be_dec)
        b_out_row = None
        if apply_b_out:
            b_out_row = const.tile([1, V], f32)
            b_ap = b_out[:]
            nc.scalar.dma_start(
                b_out_row[:],
                bass.AP(tensor=b_ap.tensor, offset=b_ap.offset,
                        ap=[[0, 1], [1, V]]))
            nc.vector.tensor_scalar(b_out_row[:], b_out_row[:], 1.0 / OS,
                                    None, OP.mult)

        # J-major normalized activations (SBUF resident for the whole loop)
        encT = const.tile([P, KJ, TC], bf16)
        decT = const.tile([P, KJ, U], bf16)
        # dec pattern pre-tiled over the 8 t-values of a supertile: dense
        # in0 for the joint build (broadcast reads slow DVE/GpSimd)
        dtile = const.tile([P, KJ, TSUP, U], bf16)
        # row-major linear terms: encW [t, v] (2 t-blocks), decB [u, v]
        encW_sb = const.tile([P, TC // P, V], bf16)
        decB_sb = const.tile([U, V], bf16)
        # assembled moving tiles for the linear matmul: rows 0:2 = enc pair,
        # rows 2:66 = decB; LBUF-deep rotation over supertiles
        L_all = const.tile([2 + U, KJ, LBUF, V], bf16)

        def layer_norm_psum(ps, rows, label, b_sb, g_sb, be_sb, hot=False,
                            ts_on_act=False):
            """LN over the free dim of psum tile ps [rows, J] -> bf16 SBUF."""
            ln16 = prep.tile([P, J], bf16, tag="ln16", name=f"ln16_{label}")
            if b_sb is not None:
                xf = prep.tile([P, J], f32, tag="lnf", name=f"lnf_{label}")
                nc.vector.tensor_add(xf[:rows], ps[:rows], b_sb[:rows])
                src = xf
            else:
                src = ps
            hp = tc.high_priority if hot else _null_ctx
            with hp():
                stats = prep.tile([P, 6], f32, tag="ln_stats", name=f"st_{label}")
                mv = prep.tile([P, 2], f32, tag="ln_mv", name=f"mv_{label}")
                nc.vector.bn_stats(out=stats[:rows], in_=src[:rows])
                nc.vector.bn_aggr(out=mv[:rows], in_=stats[:rows])
                rstd = prep.tile([P, 1], f32, tag="ln_rstd", name=f"rs_{label}")
                nc.scalar.activation(out=rstd[:rows], in_=mv[:rows, 1:2],
                                     func=AF.Sqrt, bias=eps_sb[:rows], scale=1.0)
                nc.vector.reciprocal(out=rstd[:rows], in_=rstd[:rows])
                if ts_on_act:
                    nmr = prep.tile([P, 1], f32, tag="ln_nmr", name=f"nm_{label}")
                    nc.vector.tensor_scalar(nmr[:rows], mv[:rows, 0:1],
                                            rstd[:rows], -1.0,
                                            OP.mult, OP.mult)
                    nc.scalar.activation(out=ln16[:rows], in_=src[:rows],
                                         func=AF.Identity, bias=nmr[:rows],
                                         scale=rstd[:rows])
                else:
                    nc.vector.tensor_scalar(ln16[:rows], src[:rows],
                                            mv[:rows, 0:1], rstd[:rows],
                                            OP.subtract, OP.mult)
            if g_sb is not None:
                nc.vector.tensor_mul(ln16[:rows], ln16[:rows], g_sb[:rows])
            if be_sb is not None:
                nc.vector.tensor_add(ln16[:rows], ln16[:rows], be_sb[:rows])
            return ln16

        # ---- prologue: warmup, dec + enc-tb0 projections, LNs, transposes
        warmup(16, "a")

        eps_mm0 = mpsum.tile([P, J], f32, tag="mps", name="emm_0")
        for k in range(E // P):
            nc.tensor.matmul(eps_mm0[:], xT_sb[:, k, 0:P], wenc_sb[:, k, :],
                             start=(k == 0), stop=(k == E // P - 1))
        encln0 = layer_norm_psum(eps_mm0, P, "e0", b_enc_sb, g_enc_sb,
                                 be_enc_sb, hot=True)

        dps = mpsum.tile([P, J], f32, tag="mps", name="dmm")
        for k in range(D // P):
            nc.tensor.matmul(dps[:U], dxT_sb[:, k, :], wdec_sb[:, k, :],
                             start=(k == 0), stop=(k == D // P - 1))
        decln = layer_norm_psum(dps, U, "d", b_dec_sb, g_dec_sb, be_dec_sb,
                                hot=True, ts_on_act=True)

        warmup(4, "b")

        with tc.high_priority():
            ept0 = mpsum.tile([P, KJ, P], bf16, tag="mps", name="ept_0")
            for jb in range(KJ):
                nc.tensor.transpose(ept0[:, jb, :], encln0[:, jb * P:(jb + 1) * P],
                                    ident[:])
            nc.scalar.copy(encT[:, :, 0:P], ept0[:])
            dpt = mpsum.tile([P, KJ, U], bf16, tag="mps", name="dpt")
            for jb in range(KJ):
                nc.tensor.transpose(dpt[:, jb, :], decln[:U, jb * P:(jb + 1) * P],
                                    ident[:U, :U])
            nc.scalar.copy(decT[:], dpt[:])
        # dense dec pattern on idle GpSimd (not on supertile 0's chain)
        nc.gpsimd.tensor_copy(
            dtile[:], decT[:, :, None, :].to_broadcast((P, KJ, TSUP, U)))

        warmup(4, "c")

        # ---- prologue linear GEMMs ----
        # colsum row: crow = 2b * colsum(w_lin) (+ 256*b_out)
        psC = mpsum.tile([1, V], f32, tag="mps", name="psC")
        for k in range(KJ):
            for vh in range(2):
                nc.tensor.matmul(psC[:, vh * 512:(vh + 1) * 512], onesJ[:],
                                 wlin_sb[:, k, vh * 512:(vh + 1) * 512],
                                 start=(k == 0), stop=(k == KJ - 1))
        crow_f = prep.tile([1, V], f32, tag="crow_f")
        nc.vector.tensor_scalar(crow_f[:], psC[:], c2b_sb[0:1, 0:1], None,
                                OP.mult)
        if b_out_row is not None:
            nc.vector.tensor_add(crow_f[:], crow_f[:], b_out_row[:])
        crow_sb = const.tile([1, V], bf16)
        nc.vector.tensor_copy(crow_sb[:], crow_f[:])

        # decB = dec @ w_lin + crow  (crow broadcast over u via ones-row MM)
        psD = mpsum.tile([U, V], f32, tag="mps", name="psD")
        for k in range(KJ):
            for vh in range(2):
                nc.tensor.matmul(psD[:, vh * 512:(vh + 1) * 512],
                                 decT[:, k, :],
                                 wlin_sb[:, k, vh * 512:(vh + 1) * 512],
                                 start=(k == 0), stop=False)
        for vh in range(2):
            nc.tensor.matmul(psD[:, vh * 512:(vh + 1) * 512], ones_u[:],
                             crow_sb[:, vh * 512:(vh + 1) * 512],
                             start=False, stop=True)
        nc.vector.tensor_copy(decB_sb[:], psD[:])

        # encW block 0 (t 0:128)
        def encw_gemm(blk, eng):
            psE = mpsum.tile([P, V], f32, tag="mps", name=f"psE{blk}")
            for k in range(KJ):
                for vh in range(2):
                    nc.tensor.matmul(psE[:, vh * 512:(vh + 1) * 512],
                                     encT[:, k, blk * P:(blk + 1) * P],
                                     wlin_sb[:, k, vh * 512:(vh + 1) * 512],
                                     start=(k == 0), stop=(k == KJ - 1))
            eng.tensor_copy(encW_sb[:, blk, :], psE) if eng is nc.vector \
                else eng.copy(encW_sb[:, blk, :], psE)

        encw_gemm(0, nc.scalar)

        def fill_dec(bu):
            # L dec-part fill (SBUF->SBUF DMAs, partition shift +2)
            for jb in range(KJ):
                nc.scalar.dma_start(L_all[2:2 + U, jb, bu, :], decB_sb[:])

        def asm_L(mm):
            # enc rows t0, t0+1 per row-block into L_all[0:2, jb, mm%LBUF]
            t0 = mm * TSUP
            blk, p0 = divmod(t0, P)
            for jb in range(KJ):
                nc.scalar.dma_start(
                    L_all[0:2, jb, mm % LBUF, :],
                    encW_sb[p0 + 2 * jb:p0 + 2 * jb + 2, blk, :])

        # priority order: supertile 0/1's L tiles complete before bufs 2/3
        fill_dec(0)
        asm_L(0)
        fill_dec(1)
        asm_L(1)
        fill_dec(2)
        fill_dec(3)

        # ---- main loop ----
        out_r = out[:].rearrange("(mm j p) v -> mm p j v", j=KJ, p=P)

        joints = {}
        jqs = {}

        def build_joint_q(mm, h):
            # quarter h of supertile mm's joint + residual quant; quarter h
            # covers exactly the rows of matmul row-block jb=h
            t0 = mm * TSUP
            if h == 0:
                joint = jpool.tile([P, KJ, 512], bf16, tag="joint",
                                   name=f"jnt_{mm}")
                r2 = jpool.tile([P, KJ, 512], bf16, tag="r2", name=f"r2_{mm}")
                jq = jqpool.tile([P, KJ, 512], f8, tag="jq", name=f"jq_{mm}")
                joints[mm] = (joint, r2)
                jqs[mm] = jq
            else:
                joint, r2 = joints[mm]
                jq = jqs[mm]
            jv = joint.rearrange("p k (t u) -> p k t u", u=U)
            step = TSUP // 4
            tsl = slice(h * step, (h + 1) * step)
            enc_b = encT[:, :, t0 + h * step:t0 + (h + 1) * step, None] \
                .to_broadcast((P, KJ, step, U))
            eng = nc.vector if h == 0 else nc.gpsimd
            if mm == 0:
                dec_in = decT[:, :, None, :].to_broadcast((P, KJ, step, U))
            else:
                dec_in = dtile[:, :, tsl]
            eng.tensor_tensor(jv[:, :, tsl], dec_in, enc_b, OP.add)
            sl = slice(h * step * U, (h + 1) * step * U)
            # g' = |x| - 2b = (2*relu(x) - 2b) - x  (no abs ALU on trn2 DVE):
            # relu doubling on ACT, fused subtract chain on DVE, fp8e4 out
            nc.scalar.activation(out=r2[:, :, sl], in_=joint[:, :, sl],
                                 func=AF.Relu, scale=2.0)
            nc.vector.scalar_tensor_tensor(jq[:, :, sl], r2[:, :, sl],
                                           c2b_sb[:, 0:1], joint[:, :, sl],
                                           OP.subtract, OP.subtract)

        def build_joint(mm):
            for h in range(4):
                build_joint_q(mm, h)

        def supertile(mm):
            if mm + 1 < NMM:
                build_joint(mm + 1)
            if mm + 2 < NMM:
                asm_L(mm + 2)
            jq = jqs.pop(mm)
            joints.pop(mm)
            tail = mm == NMM - 1
            for jb in range(KJ):
                pss = mpsum.tile([P, V], f32, tag="mps", name=f"ps_{mm}_{jb}")
                for kp in range(2):
                    for vh in range(2):
                        nc.tensor.matmul(
                            pss[:, vh * 512:(vh + 1) * 512],
                            jq[:, 2 * kp:2 * kp + 2, jb * P:(jb + 1) * P],
                            w8_sb[:, 2 * kp:2 * kp + 2,
                                  vh * 512:(vh + 1) * 512],
                            start=(kp == 0), stop=False, perf_mode=DR)
                for vh in range(2):
                    nc.tensor.matmul(pss[:, vh * 512:(vh + 1) * 512],
                                     s66_sb[:],
                                     L_all[:, jb, mm % LBUF,
                                           vh * 512:(vh + 1) * 512],
                                     start=False, stop=True)
                stage = opool.tile([P, V], bf16, tag="stage",
                                   name=f"st_{mm}_{jb}")
                if tail:
                    # split evictions/DMAs for a shorter serial tail
                    if jb % 2 == 0:
                        nc.scalar.copy(stage[:, :512], pss[:, :512])
                        nc.vector.tensor_copy(stage[:, 512:], pss[:, 512:])
                    else:
                        nc.vector.tensor_copy(stage[:, :512], pss[:, :512])
                        nc.scalar.copy(stage[:, 512:], pss[:, 512:])
                    nc.sync.dma_start(out_r[mm, :, jb, 0:512], stage[:, :512])
                    nc.sync.dma_start(out_r[mm, :, jb, 512:], stage[:, 512:])
                else:
                    if jb == 3:
                        nc.vector.tensor_copy(stage[:], pss[:])
                    else:
                        nc.scalar.copy(stage[:], pss[:])
                    nc.sync.dma_start(out_r[mm, :, jb], stage[:])

        build_joint(0)
        supertile(0)
        supertile(1)
        supertile(2)
        supertile(3)

        # deferred enc tb1 projection + LN (hides in main-loop slack; only
        # needed from supertile 14's asm_L(16) onward)
        eps_mm1 = mpsum.tile([P, J], f32, tag="mps", name="emm_1")
        for k in range(E // P):
            nc.tensor.matmul(eps_mm1[:], xT_sb[:, k, P:2 * P], wenc_sb[:, k, :],
                             start=(k == 0), stop=(k == E // P - 1))
        encln1 = layer_norm_psum(eps_mm1, P, "e1", b_enc_sb, g_enc_sb, be_enc_sb)

        supertile(4)
        supertile(5)
        supertile(6)

        ept1 = mpsum.tile([P, KJ, P], bf16, tag="mps", name="ept_1")
        for jb in range(KJ):
            nc.tensor.transpose(ept1[:, jb, :], encln1[:, jb * P:(jb + 1) * P],
                                ident[:])
        nc.vector.tensor_copy(encT[:, :, P:2 * P], ept1[:])

        supertile(7)
        supertile(8)
        encw_gemm(1, nc.vector)

        for mm in range(9, NMM):
            supertile(mm)

    nc.compile()
    return nc


def _host_ln(y, g, be):
    mu = y.mean(-1, keepdims=True)
    yc = y - mu
    v = (yc * yc).mean(-1, keepdims=True)
    return yc / np.sqrt(v + EPS) * g + be


def _pack(a):
    # [K, N] -> [128, K//128, N] partition-major (SBUF layout)
    a = np.asarray(a)
    return np.ascontiguousarray(
        a.reshape(a.shape[0] // 128, 128, a.shape[1]).transpose(1, 0, 2))


def prepare(inputs):
    """-> (flags, named, in_maps) for the 8 cores."""
    import ml_dtypes

    bf = ml_dtypes.bfloat16
    f8 = ml_dtypes.float8_e4m3
    enc = np.asarray(inputs["encoder_out"], dtype=np.float32)
    dec = np.asarray(inputs["decoder_out"], dtype=np.float32)
    named = {}
    for k_src in ["b_enc", "g_enc", "be_enc", "b_dec", "g_dec", "be_dec",
                  "b_out"]:
        named[k_src] = np.ascontiguousarray(
            np.asarray(inputs[k_src], dtype=np.float32))
    for k_src, k_dst in [("W_enc", "w_enc"), ("W_dec", "w_dec")]:
        named[k_dst] = _pack(
            np.asarray(inputs[k_src], dtype=np.float32).astype(bf))
    W = np.asarray(inputs["W_out"], dtype=np.float32)
    named["w8"] = _pack((W * WS).astype(f8))
    named["w_lin"] = _pack((W * WS).astype(bf))

    # one-hot selector rows: 2 t-selectors + 64 u-selectors for a 128-row
    # block (2 t values x 64 u)
    r = np.arange(P)
    s = np.zeros((2 + U, P), np.float32)
    s[0] = r // U == 0
    s[1] = r // U == 1
    s[2 + r % U, r] = 1.0
    named["s66"] = np.ascontiguousarray(s.astype(bf))

    # 2b = E[|enc+dec|] from a T-subsample (any b is exact; the optimal b
    # minimizes the fp8 residual variance)
    We = np.asarray(inputs["W_enc"], np.float32)
    Wd = np.asarray(inputs["W_dec"], np.float32)
    e_s = _host_ln(enc[:, ::16] @ We + named["b_enc"], named["g_enc"],
                   named["be_enc"])
    d_s = _host_ln(dec @ Wd + named["b_dec"], named["g_dec"], named["be_dec"])
    c2b = np.abs(e_s[:, :, None, :] + d_s[:, None, :, :]).mean()
    named["c2b"] = np.array([c2b], dtype=np.float32)

    flags = (
        bool(np.any(named["b_enc"])), not np.all(named["g_enc"] == 1.0),
        bool(np.any(named["be_enc"])),
        bool(np.any(named["b_dec"])), not np.all(named["g_dec"] == 1.0),
        bool(np.any(named["be_dec"])),
        bool(np.any(named["b_out"])),
    )

    tpc = T // (NCORES // B)      # t-rows per core
    in_maps = []
    for c in range(NCORES):
        b = c // (NCORES // B)
        t0 = (c % (NCORES // B)) * tpc
        in_maps.append({
            "enc_xT": _pack(enc[b, t0:t0 + tpc].T.astype(bf)),
            "dec_xT": _pack(dec[b].T.astype(bf)),
            **named,
        })
    return flags, named, in_maps


def kernel(**inputs):
    from concourse.bass_utils import run_bass_kernel_spmd

    flags, _, in_maps = prepare(inputs)
    if flags not in _CACHE:
        _CACHE[flags] = _build(*flags)
    nc = _CACHE[flags]

    res = run_bass_kernel_spmd(nc, in_maps, core_ids=list(range(NCORES)))
    full = np.concatenate(
        [np.asarray(res.results[c]["out"]).astype(np.float32)
         for c in range(NCORES)], axis=0)
    full *= np.float32(OS)
    return full.reshape(B, T, U, V)


# revision 33
# speedup vs baseline: 1.1676x; 1.1676x over previous
"""RNN-T JointNet fused Bass kernel for Trainium2, SPMD over 8 NeuronCores.

Reference computation (all fp32):
    enc = LN(encoder_out @ W_enc + b_enc) * g_enc + be_enc      # [B,T,J]
    dec = LN(decoder_out @ W_dec + b_dec) * g_dec + be_dec      # [B,U,J]
    joint = relu(enc[:,:,None,:] + dec[:,None,:,:])             # [B,T,U,J]
    out = joint @ W_out + b_out                                 # [B,T,U,V]

Shapes: B=4, T=512, U=64, E=D=J=512, V=1024.

Sharding: data-parallel over flattened (B,T): core c owns b = c//2,
t in [(c%2)*256, ...+256) -> 16384 contiguous output rows.

vs the bf16 baseline (253us, PE-bound: 1024 N=512 bf16 matmuls @222ns):
run the big GEMM in fp8e4 DoubleRow (2x contraction/pass). Plain fp8 of
relu(x) and W gives 3.9% rel err (gate 2e-2), so split off the optimal
affine part of the relu and compute it exactly:

    relu(x) = 0.5*x + b + g(x),  g = 0.5*|x| - b,  b = E[0.5|x|]
    out = [0.5*enc@W]_t + [0.5*dec@W + b*colsum(W) + b_out]_u + g@W

E[g^2] ~ 0.18 vs E[relu^2] ~ 1.0 so the fp8 quantization noise drops
~sqrt(5.5)x: measured 1.67e-2 end to end.

The linear terms are folded into PSUM by the PE itself (the elementwise
engines have no capacity at fp8 cadence: DVE ~100G, GpSimd ~46G elem/s,
ACT can only copy/activate): per 128-row block, one extra bf16 matmul
with stationary S66 [66,128] one-hot rows (2 t-selectors + 64
u-selectors, host-supplied constant) and moving L [66,V] = [encW rows
t0,t0+1; decB (incl b*colsum(W)+b_out)]. L tiles are assembled by tiny
SBUF->SBUF DMAs (2 rows per block from encW) two supertiles ahead;
PSUM evictions stay plain copies (ACT 3/4, DVE 1/4).

Supertile (512 rows): 4 row-blocks x [2x2 DR fp8 MMs (N=512) + 2 lin
MMs] ~ 5.8us PE; joint build (DVE 1/4, GpSimd 3/4), g-quant (single
DVE tensor_scalar: abs_max 0 -> subtract 2b -> fp8 out), both
software-pipelined one supertile ahead. Scales: w8 = fp8(W*128), w_lin
= bf16(W*128); device output is 256*out bf16, host divides by 256.
"""

import numpy as np

B, T, U = 4, 512, 64
E = D = J = 512
V = 1024
EPS = 1e-5
P = 128
NCORES = 8
TC = T * B // NCORES            # 256 t-rows per core
ROWS = TC * U                   # 16384 output rows per core
NMM = ROWS // 512               # 32 supertiles of 512 rows (8 t values)
KJ = J // P                     # 4 contraction blocks
TSUP = 512 // U                 # 8 t values per supertile
LBUF = 4                        # L-tile rotation depth
WS = 128.0                      # weight scale baked into w8/w_lin
OS = 1.0 / 256.0                # host-side output descale

_CACHE = {}


def _build(apply_b_enc, apply_g_enc, apply_be_enc,
           apply_b_dec, apply_g_dec, apply_be_dec, apply_b_out):
    import concourse.bass as bass
    import concourse.mybir as mybir
    import concourse.tile as tile
    from concourse import bacc
    from concourse.masks import make_identity

    f32 = mybir.dt.float32
    bf16 = mybir.dt.bfloat16
    f8 = mybir.dt.float8e4
    AF = mybir.ActivationFunctionType
    OP = mybir.AluOpType
    DR = mybir.MatmulPerfMode.DoubleRow

    nc = bacc.Bacc(target_bir_lowering=False)

    enc_xT = nc.dram_tensor("enc_xT", [P, E // P, TC], bf16, kind="ExternalInput")
    dec_xT = nc.dram_tensor("dec_xT", [P, D // P, U], bf16, kind="ExternalInput")
    w_enc = nc.dram_tensor("w_enc", [P, E // P, J], bf16, kind="ExternalInput")
    w_dec = nc.dram_tensor("w_dec", [P, D // P, J], bf16, kind="ExternalInput")
    w8 = nc.dram_tensor("w8", [P, KJ, V], f8, kind="ExternalInput")
    w_lin = nc.dram_tensor("w_lin", [P, KJ, V], bf16, kind="ExternalInput")
    s66 = nc.dram_tensor("s66", [2 + U, P], bf16, kind="ExternalInput")
    c2b = nc.dram_tensor("c2b", [1], f32, kind="ExternalInput")
    b_enc = nc.dram_tensor("b_enc", [J], f32, kind="ExternalInput")
    g_enc = nc.dram_tensor("g_enc", [J], f32, kind="ExternalInput")
    be_enc = nc.dram_tensor("be_enc", [J], f32, kind="ExternalInput")
    b_dec = nc.dram_tensor("b_dec", [J], f32, kind="ExternalInput")
    g_dec = nc.dram_tensor("g_dec", [J], f32, kind="ExternalInput")
    be_dec = nc.dram_tensor("be_dec", [J], f32, kind="ExternalInput")
    b_out = nc.dram_tensor("b_out", [V], f32, kind="ExternalInput")
    out = nc.dram_tensor("out", [ROWS, V], bf16, kind="ExternalOutput")

    def bcast_row(dram_vec, n):
        ap = dram_vec[:]
        return bass.AP(tensor=ap.tensor, offset=ap.offset,
                       ap=[[0, P], [1, n]])

    from contextlib import ExitStack, contextmanager

    @contextmanager
    def _null_ctx():
        yield

    with tile.TileContext(nc) as tc, ExitStack() as ctx:
        const = ctx.enter_context(tc.tile_pool(name="const", bufs=1))
        prep = ctx.enter_context(tc.tile_pool(name="prep", bufs=2))
        jpool = ctx.enter_context(tc.tile_pool(name="jpool", bufs=3))
        jqpool = ctx.enter_context(tc.tile_pool(name="jqpool", bufs=3))
        opool = ctx.enter_context(tc.tile_pool(name="opool", bufs=10))
        mpsum = ctx.enter_context(tc.tile_pool(name="mpsum", bufs=4, space="PSUM"))

        # ---- input DMAs, priority-ordered on the SP queue ----
        xT_sb = prep.tile([P, E // P, TC], bf16, tag="xT_sb")
        nc.sync.dma_start(xT_sb[:], enc_xT[:])
        wenc_sb = const.tile([P, E // P, J], bf16)
        nc.sync.dma_start(wenc_sb[:], w_enc[:])
        dxT_sb = prep.tile([P, D // P, U], bf16, tag="dxT_sb")
        nc.sync.dma_start(dxT_sb[:], dec_xT[:])
        wdec_sb = const.tile([P, D // P, J], bf16)
        nc.sync.dma_start(wdec_sb[:], w_dec[:])
        wlin_sb = const.tile([P, KJ, V], bf16)
        nc.sync.dma_start(wlin_sb[:], w_lin[:])
        w8_sb = const.tile([P, KJ, V], f8)
        nc.sync.dma_start(w8_sb[:], w8[:])

        ident = const.tile([P, P], bf16)
        make_identity(nc, ident)

        warm_src = const.tile([P, 512], bf16)
        nc.vector.memset(warm_src[:], 0.0)
        onesJ = const.tile([P, 1], bf16)
        nc.vector.memset(onesJ[:], 1.0)
        ones_u = const.tile([1, U], bf16)
        nc.vector.memset(ones_u[:], 1.0)

        def warmup(n, label):
            for i in range(n):
                wps = mpsum.tile([P, 512], f32, tag="mps", name=f"w{label}_{i}")
                nc.tensor.matmul(wps, warm_src[:, :P], warm_src[:],
                                 start=True, stop=True)

        eps_sb = const.tile([P, 1], f32)
        nc.vector.memset(eps_sb[:], EPS)
        zero_sb = const.tile([P, 1], f32)
        nc.vector.memset(zero_sb[:], 0.0)

        def load_vec(vec, n, enabled):
            if not enabled:
                return None
            t = const.tile([P, n], f32)
            nc.scalar.dma_start(t[:], bcast_row(vec, n))
            return t

        s66_sb = const.tile([2 + U, P], bf16)
        nc.scalar.dma_start(s66_sb[:], s66[:])
        c2b_sb = const.tile([P, 1], f32)
        nc.scalar.dma_start(c2b_sb[:], bcast_row(c2b, 1))
        b_enc_sb = load_vec(b_enc, J, apply_b_enc)
        g_enc_sb = load_vec(g_enc, J, apply_g_enc)
        be_enc_sb = load_vec(be_enc, J, apply_be_enc)
        b_dec_sb = load_vec(b_dec, J, apply_b_dec)
        g_dec_sb = load_vec(g_dec, J, apply_g_dec)
        be_dec_sb = load_vec(be_dec, J, apply_be_dec)
        b_out_row = None
        if apply_b_out:
            b_out_row = const.tile([1, V], f32)
            b_ap = b_out[:]
            nc.scalar.dma_start(
                b_out_row[:],
                bass.AP(tensor=b_ap.tensor, offset=b_ap.offset,
                        ap=[[0, 1], [1, V]]))
            nc.vector.tensor_scalar(b_out_row[:], b_out_row[:], 1.0 / OS,
                                    None, OP.mult)

        # J-major normalized activations (SBUF resident for the whole loop)
        encT = const.tile([P, KJ, TC], bf16)
        decT = const.tile([P, KJ, U], bf16)
        # row-major linear terms: encW [t, v] (2 t-blocks), decB [u, v]
        encW_sb = const.tile([P, TC // P, V], bf16)
        decB_sb = const.tile([U, V], bf16)
        # assembled moving tiles for the linear matmul: rows 0:2 = enc pair,
        # rows 2:66 = decB; LBUF-deep rotation over supertiles
        L_all = const.tile([2 + U, KJ, LBUF, V], bf16)

        def layer_norm_psum(ps, rows, label, b_sb, g_sb, be_sb, hot=False,
                            ts_on_act=False):
            """LN over the free dim of psum tile ps [rows, J] -> bf16 SBUF."""
            ln16 = prep.tile([P, J], bf16, tag="ln16", name=f"ln16_{label}")
            if b_sb is not None:
                xf = prep.tile([P, J], f32, tag="lnf", name=f"lnf_{label}")
                nc.vector.tensor_add(xf[:rows], ps[:rows], b_sb[:rows])
                src = xf
            else:
                src = ps
            hp = tc.high_priority if hot else _null_ctx
            with hp():
                stats = prep.tile([P, 6], f32, tag="ln_stats", name=f"st_{label}")
                mv = prep.tile([P, 2], f32, tag="ln_mv", name=f"mv_{label}")
                nc.vector.bn_stats(out=stats[:rows], in_=src[:rows])
                nc.vector.bn_aggr(out=mv[:rows], in_=stats[:rows])
                rstd = prep.tile([P, 1], f32, tag="ln_rstd", name=f"rs_{label}")
                nc.scalar.activation(out=rstd[:rows], in_=mv[:rows, 1:2],
                                     func=AF.Sqrt, bias=eps_sb[:rows], scale=1.0)
                nc.vector.reciprocal(out=rstd[:rows], in_=rstd[:rows])
                if ts_on_act:
                    nmr = prep.tile([P, 1], f32, tag="ln_nmr", name=f"nm_{label}")
                    nc.vector.tensor_scalar(nmr[:rows], mv[:rows, 0:1],
                                            rstd[:rows], -1.0,
                                            OP.mult, OP.mult)
                    nc.scalar.activation(out=ln16[:rows], in_=src[:rows],
                                         func=AF.Identity, bias=nmr[:rows],
                                         scale=rstd[:rows])
                else:
                    nc.vector.tensor_scalar(ln16[:rows], src[:rows],
                                            mv[:rows, 0:1], rstd[:rows],
                                            OP.subtract, OP.mult)
            if g_sb is not None:
                nc.vector.tensor_mul(ln16[:rows], ln16[:rows], g_sb[:rows])
            if be_sb is not None:
                nc.vector.tensor_add(ln16[:rows], ln16[:rows], be_sb[:rows])
            return ln16

        # ---- prologue: warmup, dec + enc-tb0 projections, LNs, transposes
        warmup(16, "a")

        eps_mm0 = mpsum.tile([P, J], f32, tag="mps", name="emm_0")
        for k in range(E // P):
            nc.tensor.matmul(eps_mm0[:], xT_sb[:, k, 0:P], wenc_sb[:, k, :],
                             start=(k == 0), stop=(k == E // P - 1))
        encln0 = layer_norm_psum(eps_mm0, P, "e0", b_enc_sb, g_enc_sb,
                                 be_enc_sb, hot=True)

        dps = mpsum.tile([P, J], f32, tag="mps", name="dmm")
        for k in range(D // P):
            nc.tensor.matmul(dps[:U], dxT_sb[:, k, :], wdec_sb[:, k, :],
                             start=(k == 0), stop=(k == D // P - 1))
        decln = layer_norm_psum(dps, U, "d", b_dec_sb, g_dec_sb, be_dec_sb,
                                hot=True, ts_on_act=True)

        warmup(4, "b")

        with tc.high_priority():
            ept0 = mpsum.tile([P, KJ, P], bf16, tag="mps", name="ept_0")
            for jb in range(KJ):
                nc.tensor.transpose(ept0[:, jb, :], encln0[:, jb * P:(jb + 1) * P],
                                    ident[:])
            nc.scalar.copy(encT[:, :, 0:P], ept0[:])
            dpt = mpsum.tile([P, KJ, U], bf16, tag="mps", name="dpt")
            for jb in range(KJ):
                nc.tensor.transpose(dpt[:, jb, :], decln[:U, jb * P:(jb + 1) * P],
                                    ident[:U, :U])
            nc.scalar.copy(decT[:], dpt[:])

        warmup(4, "c")

        # ---- prologue linear GEMMs ----
        # colsum row: crow = 2b * colsum(w_lin) (+ 256*b_out)
        psC = mpsum.tile([1, V], f32, tag="mps", name="psC")
        for k in range(KJ):
            for vh in range(2):
                nc.tensor.matmul(psC[:, vh * 512:(vh + 1) * 512], onesJ[:],
                                 wlin_sb[:, k, vh * 512:(vh + 1) * 512],
                                 start=(k == 0), stop=(k == KJ - 1))
        crow_f = prep.tile([1, V], f32, tag="crow_f")
        nc.vector.tensor_scalar(crow_f[:], psC[:], c2b_sb[0:1, 0:1], None,
                                OP.mult)
        if b_out_row is not None:
            nc.vector.tensor_add(crow_f[:], crow_f[:], b_out_row[:])
        crow_sb = const.tile([1, V], bf16)
        nc.vector.tensor_copy(crow_sb[:], crow_f[:])

        # decB = dec @ w_lin + crow  (crow broadcast over u via ones-row MM)
        psD = mpsum.tile([U, V], f32, tag="mps", name="psD")
        for k in range(KJ):
            for vh in range(2):
                nc.tensor.matmul(psD[:, vh * 512:(vh + 1) * 512],
                                 decT[:, k, :],
                                 wlin_sb[:, k, vh * 512:(vh + 1) * 512],
                                 start=(k == 0), stop=False)
        for vh in range(2):
            nc.tensor.matmul(psD[:, vh * 512:(vh + 1) * 512], ones_u[:],
                             crow_sb[:, vh * 512:(vh + 1) * 512],
                             start=False, stop=True)
        nc.vector.tensor_copy(decB_sb[:], psD[:])

        # encW block 0 (t 0:128)
        def encw_gemm(blk, eng):
            psE = mpsum.tile([P, V], f32, tag="mps", name=f"psE{blk}")
            for k in range(KJ):
                for vh in range(2):
                    nc.tensor.matmul(psE[:, vh * 512:(vh + 1) * 512],
                                     encT[:, k, blk * P:(blk + 1) * P],
                                     wlin_sb[:, k, vh * 512:(vh + 1) * 512],
                                     start=(k == 0), stop=(k == KJ - 1))
            eng.tensor_copy(encW_sb[:, blk, :], psE) if eng is nc.vector \
                else eng.copy(encW_sb[:, blk, :], psE)

        encw_gemm(0, nc.scalar)

        # L dec-part fill (SBUF->SBUF DMAs, partition shift +2), all bufs
        for jb in range(KJ):
            for bu in range(LBUF):
                nc.scalar.dma_start(L_all[2:2 + U, jb, bu, :], decB_sb[:])

        def asm_L(mm):
            # enc rows t0, t0+1 per row-block into L_all[0:2, jb, mm%LBUF]
            t0 = mm * TSUP
            blk, p0 = divmod(t0, P)
            for jb in range(KJ):
                nc.scalar.dma_start(
                    L_all[0:2, jb, mm % LBUF, :],
                    encW_sb[p0 + 2 * jb:p0 + 2 * jb + 2, blk, :])

        # ---- main loop ----
        out_r = out[:].rearrange("(mm j p) v -> mm p j v", j=KJ, p=P)

        joints = {}
        jqs = {}

        def build_joint_q(mm, h):
            # quarter h of supertile mm's joint + residual quant; quarter h
            # covers exactly the rows of matmul row-block jb=h
            t0 = mm * TSUP
            if h == 0:
                joint = jpool.tile([P, KJ, 512], bf16, tag="joint",
                                   name=f"jnt_{mm}")
                r2 = jpool.tile([P, KJ, 512], bf16, tag="r2", name=f"r2_{mm}")
                jq = jqpool.tile([P, KJ, 512], f8, tag="jq", name=f"jq_{mm}")
                joints[mm] = (joint, r2)
                jqs[mm] = jq
            else:
                joint, r2 = joints[mm]
                jq = jqs[mm]
            jv = joint.rearrange("p k (t u) -> p k t u", u=U)
            step = TSUP // 4
            tsl = slice(h * step, (h + 1) * step)
            enc_b = encT[:, :, t0 + h * step:t0 + (h + 1) * step, None] \
                .to_broadcast((P, KJ, step, U))
            dec_b = decT[:, :, None, :].to_broadcast((P, KJ, step, U))
            # all quarters on GpSimd: DVE is the max-loaded engine (quant
            # stt + evictions); GpSimd has ~2us/supertile of slack
            nc.gpsimd.tensor_tensor(jv[:, :, tsl], dec_b, enc_b, OP.add)
            sl = slice(h * step * U, (h + 1) * step * U)
            # g' = |x| - 2b = (2*relu(x) - 2b) - x  (no abs ALU on trn2 DVE):
            # relu doubling on ACT, fused subtract chain on DVE, fp8e4 out
            nc.scalar.activation(out=r2[:, :, sl], in_=joint[:, :, sl],
                                 func=AF.Relu, scale=2.0)
            nc.vector.scalar_tensor_tensor(jq[:, :, sl], r2[:, :, sl],
                                           c2b_sb[:, 0:1], joint[:, :, sl],
                                           OP.subtract, OP.subtract)

        def build_joint(mm):
            for h in range(4):
                build_joint_q(mm, h)

        def supertile(mm):
            if mm + 1 < NMM:
                build_joint(mm + 1)
            if mm + 2 < NMM:
                asm_L(mm + 2)
            jq = jqs.pop(mm)
            joints.pop(mm)
            tail = mm == NMM - 1
            for jb in range(KJ):
                pss = mpsum.tile([P, V], f32, tag="mps", name=f"ps_{mm}_{jb}")
                for kp in range(2):
                    for vh in range(2):
                        nc.tensor.matmul(
                            pss[:, vh * 512:(vh + 1) * 512],
                            jq[:, 2 * kp:2 * kp + 2, jb * P:(jb + 1) * P],
                            w8_sb[:, 2 * kp:2 * kp + 2,
                                  vh * 512:(vh + 1) * 512],
                            start=(kp == 0), stop=False, perf_mode=DR)
                for vh in range(2):
                    nc.tensor.matmul(pss[:, vh * 512:(vh + 1) * 512],
                                     s66_sb[:],
                                     L_all[:, jb, mm % LBUF,
                                           vh * 512:(vh + 1) * 512],
                                     start=False, stop=True)
                stage = opool.tile([P, V], bf16, tag="stage",
                                   name=f"st_{mm}_{jb}")
                if tail:
                    # split evictions/DMAs for a shorter serial tail
                    if jb % 2 == 0:
                        nc.scalar.copy(stage[:, :512], pss[:, :512])
                        nc.vector.tensor_copy(stage[:, 512:], pss[:, 512:])
                    else:
                        nc.vector.tensor_copy(stage[:, :512], pss[:, :512])
                        nc.scalar.copy(stage[:, 512:], pss[:, 512:])
                    nc.sync.dma_start(out_r[mm, :, jb, 0:512], stage[:, :512])
                    nc.sync.dma_start(out_r[mm, :, jb, 512:], stage[:, 512:])
                else:
                    if jb == 3:
                        nc.vector.tensor_copy(stage[:], pss[:])
                    else:
                        nc.scalar.copy(stage[:], pss[:])
                    nc.sync.dma_start(out_r[mm, :, jb], stage[:])

        build_joint(0)
        asm_L(0)
        asm_L(1)
        supertile(0)
        supertile(1)

        # deferred enc tb1 projection + LN (hides in main-loop slack)
        eps_mm1 = mpsum.tile([P, J], f32, tag="mps", name="emm_1")
        for k in range(E // P):
            nc.tensor.matmul(eps_mm1[:], xT_sb[:, k, P:2 * P], wenc_sb[:, k, :],
                             start=(k == 0), stop=(k == E // P - 1))
        encln1 = layer_norm_psum(eps_mm1, P, "e1", b_enc_sb, g_enc_sb, be_enc_sb)

        supertile(2)
        supertile(3)

        ept1 = mpsum.tile([P, KJ, P], bf16, tag="mps", name="ept_1")
        for jb in range(KJ):
            nc.tensor.transpose(ept1[:, jb, :], encln1[:, jb * P:(jb + 1) * P],
                                ident[:])
        nc.vector.tensor_copy(encT[:, :, P:2 * P], ept1[:])

        supertile(4)
        encw_gemm(1, nc.vector)

        for mm in range(5, NMM):
            supertile(mm)

    nc.compile()
    return nc


def _host_ln(y, g, be):
    mu = y.mean(-1, keepdims=True)
    yc = y - mu
    v = (yc * yc).mean(-1, keepdims=True)
    return yc / np.sqrt(v + EPS) * g + be


def _pack(a):
    # [K, N] -> [128, K//128, N] partition-major (SBUF layout)
    a = np.asarray(a)
    return np.ascontiguousarray(
        a.reshape(a.shape[0] // 128, 128, a.shape[1]).transpose(1, 0, 2))


def prepare(inputs):
    """-> (flags, named, in_maps) for the 8 cores."""
    import ml_dtypes

    bf = ml_dtypes.bfloat16
    f8 = ml_dtypes.float8_e4m3
    enc = np.asarray(inputs["encoder_out"], dtype=np.float32)
    dec = np.asarray(inputs["decoder_out"], dtype=np.float32)
    named = {}
    for k_src in ["b_enc", "g_enc", "be_enc", "b_dec", "g_dec", "be_dec",
                  "b_out"]:
        named[k_src] = np.ascontiguousarray(
            np.asarray(inputs[k_src], dtype=np.float32))
    for k_src, k_dst in [("W_enc", "w_enc"), ("W_dec", "w_dec")]:
        named[k_dst] = _pack(
            np.asarray(inputs[k_src], dtype=np.float32).astype(bf))
    W = np.asarray(inputs["W_out"], dtype=np.float32)
    named["w8"] = _pack((W * WS).astype(f8))
    named["w_lin"] = _pack((W * WS).astype(bf))

    # one-hot selector rows: 2 t-selectors + 64 u-selectors for a 128-row
    # block (2 t values x 64 u)
    r = np.arange(P)
    s = np.zeros((2 + U, P), np.float32)
    s[0] = r // U == 0
    s[1] = r // U == 1
    s[2 + r % U, r] = 1.0
    named["s66"] = np.ascontiguousarray(s.astype(bf))

    # 2b = E[|enc+dec|] from a T-subsample (any b is exact; the optimal b
    # minimizes the fp8 residual variance)
    We = np.asarray(inputs["W_enc"], np.float32)
    Wd = np.asarray(inputs["W_dec"], np.float32)
    e_s = _host_ln(enc[:, ::16] @ We + named["b_enc"], named["g_enc"],
                   named["be_enc"])
    d_s = _host_ln(dec @ Wd + named["b_dec"], named["g_dec"], named["be_dec"])
    c2b = np.abs(e_s[:, :, None, :] + d_s[:, None, :, :]).mean()
    named["c2b"] = np.array([c2b], dtype=np.float32)

    flags = (
        bool(np.any(named["b_enc"])), not np.all(named["g_enc"] == 1.0),
        bool(np.any(named["be_enc"])),
        bool(np.any(named["b_dec"])), not np.all(named["g_dec"] == 1.0),
        bool(np.any(named["be_dec"])),
        bool(np.any(named["b_out"])),
    )

    tpc = T // (NCORES // B)      # t-rows per core
    in_maps = []
    for c in range(NCORES):
        b = c // (NCORES // B)
        t0 = (c % (NCORES // B)) * tpc
        in_maps.append({
            "enc_xT": _pack(enc[b, t0:t0 + tpc].T.astype(bf)),
            "dec_xT": _pack(dec[b].T.astype(bf)),
            **named,
        })
    return flags, named, in_maps


def kernel(**inputs):
    from concourse.bass_utils import run_bass_kernel_spmd

    flags, _, in_maps = prepare(inputs)
    if flags not in _CACHE:
        _CACHE[flags] = _build(*flags)
    nc = _CACHE[flags]

    res = run_bass_kernel_spmd(nc, in_maps, core_ids=list(range(NCORES)))
    full = np.concatenate(
        [np.asarray(res.results[c]["out"]).astype(np.float32)
         for c in range(NCORES)], axis=0)
    full *= np.float32(OS)
    return full.reshape(B, T, U, V)


# revision 35
# speedup vs baseline: 1.2002x; 1.0279x over previous
"""RNN-T JointNet fused Bass kernel for Trainium2, SPMD over 8 NeuronCores.

Reference computation (all fp32):
    enc = LN(encoder_out @ W_enc + b_enc) * g_enc + be_enc      # [B,T,J]
    dec = LN(decoder_out @ W_dec + b_dec) * g_dec + be_dec      # [B,U,J]
    joint = relu(enc[:,:,None,:] + dec[:,None,:,:])             # [B,T,U,J]
    out = joint @ W_out + b_out                                 # [B,T,U,V]

Shapes: B=4, T=512, U=64, E=D=J=512, V=1024.

Sharding: data-parallel over flattened (B,T): core c owns b = c//2,
t in [(c%2)*256, ...+256) -> 16384 contiguous output rows.

vs the bf16 baseline (253us, PE-bound: 1024 N=512 bf16 matmuls @222ns):
run the big GEMM in fp8e4 DoubleRow (2x contraction/pass). Plain fp8 of
relu(x) and W gives 3.9% rel err (gate 2e-2), so split off the optimal
affine part of the relu and compute it exactly:

    relu(x) = 0.5*x + b + g(x),  g = 0.5*|x| - b,  b = E[0.5|x|]
    out = [0.5*enc@W]_t + [0.5*dec@W + b*colsum(W) + b_out]_u + g@W

E[g^2] ~ 0.18 vs E[relu^2] ~ 1.0 so the fp8 quantization noise drops
~sqrt(5.5)x: measured 1.67e-2 end to end.

The linear terms are folded into PSUM by the PE itself (the elementwise
engines have no capacity at fp8 cadence: DVE ~100G, GpSimd ~46G elem/s,
ACT can only copy/activate): per 128-row block, one extra bf16 matmul
with stationary S66 [66,128] one-hot rows (2 t-selectors + 64
u-selectors, host-supplied constant) and moving L [66,V] = [encW rows
t0,t0+1; decB (incl b*colsum(W)+b_out)]. L tiles are assembled by tiny
SBUF->SBUF DMAs (2 rows per block from encW) two supertiles ahead;
PSUM evictions stay plain copies (ACT 3/4, DVE 1/4).

Supertile (512 rows): 4 row-blocks x [2x2 DR fp8 MMs (N=512) + 2 lin
MMs] ~ 5.8us PE; joint build (DVE 1/4, GpSimd 3/4), g-quant (single
DVE tensor_scalar: abs_max 0 -> subtract 2b -> fp8 out), both
software-pipelined one supertile ahead. Scales: w8 = fp8(W*128), w_lin
= bf16(W*128); device output is 256*out bf16, host divides by 256.
"""

import numpy as np

B, T, U = 4, 512, 64
E = D = J = 512
V = 1024
EPS = 1e-5
P = 128
NCORES = 8
TC = T * B // NCORES            # 256 t-rows per core
ROWS = TC * U                   # 16384 output rows per core
NMM = ROWS // 512               # 32 supertiles of 512 rows (8 t values)
KJ = J // P                     # 4 contraction blocks
TSUP = 512 // U                 # 8 t values per supertile
LBUF = 4                        # L-tile rotation depth
WS = 128.0                      # weight scale baked into w8/w_lin
OS = 1.0 / 256.0                # host-side output descale

_CACHE = {}


def _build(apply_b_enc, apply_g_enc, apply_be_enc,
           apply_b_dec, apply_g_dec, apply_be_dec, apply_b_out):
    import concourse.bass as bass
    import concourse.mybir as mybir
    import concourse.tile as tile
    from concourse import bacc
    from concourse.masks import make_identity

    f32 = mybir.dt.float32
    bf16 = mybir.dt.bfloat16
    f8 = mybir.dt.float8e4
    AF = mybir.ActivationFunctionType
    OP = mybir.AluOpType
    DR = mybir.MatmulPerfMode.DoubleRow

    nc = bacc.Bacc(target_bir_lowering=False)

    enc_xT = nc.dram_tensor("enc_xT", [P, E // P, TC], bf16, kind="ExternalInput")
    dec_xT = nc.dram_tensor("dec_xT", [P, D // P, U], bf16, kind="ExternalInput")
    w_enc = nc.dram_tensor("w_enc", [P, E // P, J], bf16, kind="ExternalInput")
    w_dec = nc.dram_tensor("w_dec", [P, D // P, J], bf16, kind="ExternalInput")
    w8 = nc.dram_tensor("w8", [P, KJ, V], f8, kind="ExternalInput")
    w_lin = nc.dram_tensor("w_lin", [P, KJ, V], bf16, kind="ExternalInput")
    s66 = nc.dram_tensor("s66", [2 + U, P], bf16, kind="ExternalInput")
    c2b = nc.dram_tensor("c2b", [1], f32, kind="ExternalInput")
    b_enc = nc.dram_tensor("b_enc", [J], f32, kind="ExternalInput")
    g_enc = nc.dram_tensor("g_enc", [J], f32, kind="ExternalInput")
    be_enc = nc.dram_tensor("be_enc", [J], f32, kind="ExternalInput")
    b_dec = nc.dram_tensor("b_dec", [J], f32, kind="ExternalInput")
    g_dec = nc.dram_tensor("g_dec", [J], f32, kind="ExternalInput")
    be_dec = nc.dram_tensor("be_dec", [J], f32, kind="ExternalInput")
    b_out = nc.dram_tensor("b_out", [V], f32, kind="ExternalInput")
    out = nc.dram_tensor("out", [ROWS, V], bf16, kind="ExternalOutput")

    def bcast_row(dram_vec, n):
        ap = dram_vec[:]
        return bass.AP(tensor=ap.tensor, offset=ap.offset,
                       ap=[[0, P], [1, n]])

    from contextlib import ExitStack, contextmanager

    @contextmanager
    def _null_ctx():
        yield

    with tile.TileContext(nc) as tc, ExitStack() as ctx:
        const = ctx.enter_context(tc.tile_pool(name="const", bufs=1))
        prep = ctx.enter_context(tc.tile_pool(name="prep", bufs=2))
        jpool = ctx.enter_context(tc.tile_pool(name="jpool", bufs=3))
        jqpool = ctx.enter_context(tc.tile_pool(name="jqpool", bufs=3))
        opool = ctx.enter_context(tc.tile_pool(name="opool", bufs=10))
        mpsum = ctx.enter_context(tc.tile_pool(name="mpsum", bufs=4, space="PSUM"))

        # ---- input DMAs, priority-ordered on the SP queue ----
        xT_sb = prep.tile([P, E // P, TC], bf16, tag="xT_sb")
        nc.sync.dma_start(xT_sb[:], enc_xT[:])
        wenc_sb = const.tile([P, E // P, J], bf16)
        nc.sync.dma_start(wenc_sb[:], w_enc[:])
        dxT_sb = prep.tile([P, D // P, U], bf16, tag="dxT_sb")
        nc.sync.dma_start(dxT_sb[:], dec_xT[:])
        wdec_sb = const.tile([P, D // P, J], bf16)
        nc.sync.dma_start(wdec_sb[:], w_dec[:])
        wlin_sb = const.tile([P, KJ, V], bf16)
        nc.sync.dma_start(wlin_sb[:], w_lin[:])
        w8_sb = const.tile([P, KJ, V], f8)
        nc.sync.dma_start(w8_sb[:], w8[:])

        ident = const.tile([P, P], bf16)
        make_identity(nc, ident)

        warm_src = const.tile([P, 512], bf16)
        nc.vector.memset(warm_src[:], 0.0)
        onesJ = const.tile([P, 1], bf16)
        nc.vector.memset(onesJ[:], 1.0)
        ones_u = const.tile([1, U], bf16)
        nc.vector.memset(ones_u[:], 1.0)

        def warmup(n, label):
            for i in range(n):
                wps = mpsum.tile([P, 512], f32, tag="mps", name=f"w{label}_{i}")
                nc.tensor.matmul(wps, warm_src[:, :P], warm_src[:],
                                 start=True, stop=True)

        eps_sb = const.tile([P, 1], f32)
        nc.vector.memset(eps_sb[:], EPS)
        zero_sb = const.tile([P, 1], f32)
        nc.vector.memset(zero_sb[:], 0.0)

        def load_vec(vec, n, enabled):
            if not enabled:
                return None
            t = const.tile([P, n], f32)
            nc.scalar.dma_start(t[:], bcast_row(vec, n))
            return t

        s66_sb = const.tile([2 + U, P], bf16)
        nc.scalar.dma_start(s66_sb[:], s66[:])
        c2b_sb = const.tile([P, 1], f32)
        nc.scalar.dma_start(c2b_sb[:], bcast_row(c2b, 1))
        b_enc_sb = load_vec(b_enc, J, apply_b_enc)
        g_enc_sb = load_vec(g_enc, J, apply_g_enc)
        be_enc_sb = load_vec(be_enc, J, apply_be_enc)
        b_dec_sb = load_vec(b_dec, J, apply_b_dec)
        g_dec_sb = load_vec(g_dec, J, apply_g_dec)
        be_dec_sb = load_vec(be_dec, J, apply_be_dec)
        b_out_row = None
        if apply_b_out:
            b_out_row = const.tile([1, V], f32)
            b_ap = b_out[:]
            nc.scalar.dma_start(
                b_out_row[:],
                bass.AP(tensor=b_ap.tensor, offset=b_ap.offset,
                        ap=[[0, 1], [1, V]]))
            nc.vector.tensor_scalar(b_out_row[:], b_out_row[:], 1.0 / OS,
                                    None, OP.mult)

        # J-major normalized activations (SBUF resident for the whole loop)
        encT = const.tile([P, KJ, TC], bf16)
        decT = const.tile([P, KJ, U], bf16)
        # row-major linear terms: encW [t, v] (2 t-blocks), decB [u, v]
        encW_sb = const.tile([P, TC // P, V], bf16)
        decB_sb = const.tile([U, V], bf16)
        # assembled moving tiles for the linear matmul: rows 0:2 = enc pair,
        # rows 2:66 = decB; LBUF-deep rotation over supertiles
        L_all = const.tile([2 + U, KJ, LBUF, V], bf16)

        def layer_norm_psum(ps, rows, label, b_sb, g_sb, be_sb, hot=False,
                            ts_on_act=False):
            """LN over the free dim of psum tile ps [rows, J] -> bf16 SBUF."""
            ln16 = prep.tile([P, J], bf16, tag="ln16", name=f"ln16_{label}")
            if b_sb is not None:
                xf = prep.tile([P, J], f32, tag="lnf", name=f"lnf_{label}")
                nc.vector.tensor_add(xf[:rows], ps[:rows], b_sb[:rows])
                src = xf
            else:
                src = ps
            hp = tc.high_priority if hot else _null_ctx
            with hp():
                stats = prep.tile([P, 6], f32, tag="ln_stats", name=f"st_{label}")
                mv = prep.tile([P, 2], f32, tag="ln_mv", name=f"mv_{label}")
                nc.vector.bn_stats(out=stats[:rows], in_=src[:rows])
                nc.vector.bn_aggr(out=mv[:rows], in_=stats[:rows])
                rstd = prep.tile([P, 1], f32, tag="ln_rstd", name=f"rs_{label}")
                nc.scalar.activation(out=rstd[:rows], in_=mv[:rows, 1:2],
                                     func=AF.Sqrt, bias=eps_sb[:rows], scale=1.0)
                nc.vector.reciprocal(out=rstd[:rows], in_=rstd[:rows])
                if ts_on_act:
                    nmr = prep.tile([P, 1], f32, tag="ln_nmr", name=f"nm_{label}")
                    nc.vector.tensor_scalar(nmr[:rows], mv[:rows, 0:1],
                                            rstd[:rows], -1.0,
                                            OP.mult, OP.mult)
                    nc.scalar.activation(out=ln16[:rows], in_=src[:rows],
                                         func=AF.Identity, bias=nmr[:rows],
                                         scale=rstd[:rows])
                else:
                    nc.vector.tensor_scalar(ln16[:rows], src[:rows],
                                            mv[:rows, 0:1], rstd[:rows],
                                            OP.subtract, OP.mult)
            if g_sb is not None:
                nc.vector.tensor_mul(ln16[:rows], ln16[:rows], g_sb[:rows])
            if be_sb is not None:
                nc.vector.tensor_add(ln16[:rows], ln16[:rows], be_sb[:rows])
            return ln16

        # ---- prologue: warmup, dec + enc-tb0 projections, LNs, transposes
        warmup(16, "a")

        eps_mm0 = mpsum.tile([P, J], f32, tag="mps", name="emm_0")
        for k in range(E // P):
            nc.tensor.matmul(eps_mm0[:], xT_sb[:, k, 0:P], wenc_sb[:, k, :],
                             start=(k == 0), stop=(k == E // P - 1))
        encln0 = layer_norm_psum(eps_mm0, P, "e0", b_enc_sb, g_enc_sb,
                                 be_enc_sb, hot=True)

        dps = mpsum.tile([P, J], f32, tag="mps", name="dmm")
        for k in range(D // P):
            nc.tensor.matmul(dps[:U], dxT_sb[:, k, :], wdec_sb[:, k, :],
                             start=(k == 0), stop=(k == D // P - 1))
        decln = layer_norm_psum(dps, U, "d", b_dec_sb, g_dec_sb, be_dec_sb,
                                hot=True, ts_on_act=True)

        warmup(4, "b")

        with tc.high_priority():
            ept0 = mpsum.tile([P, KJ, P], bf16, tag="mps", name="ept_0")
            for jb in range(KJ):
                nc.tensor.transpose(ept0[:, jb, :], encln0[:, jb * P:(jb + 1) * P],
                                    ident[:])
            nc.scalar.copy(encT[:, :, 0:P], ept0[:])
            dpt = mpsum.tile([P, KJ, U], bf16, tag="mps", name="dpt")
            for jb in range(KJ):
                nc.tensor.transpose(dpt[:, jb, :], decln[:U, jb * P:(jb + 1) * P],
                                    ident[:U, :U])
            nc.scalar.copy(decT[:], dpt[:])

        warmup(4, "c")

        # ---- prologue linear GEMMs ----
        # colsum row: crow = 2b * colsum(w_lin) (+ 256*b_out)
        psC = mpsum.tile([1, V], f32, tag="mps", name="psC")
        for k in range(KJ):
            for vh in range(2):
                nc.tensor.matmul(psC[:, vh * 512:(vh + 1) * 512], onesJ[:],
                                 wlin_sb[:, k, vh * 512:(vh + 1) * 512],
                                 start=(k == 0), stop=(k == KJ - 1))
        crow_f = prep.tile([1, V], f32, tag="crow_f")
        nc.vector.tensor_scalar(crow_f[:], psC[:], c2b_sb[0:1, 0:1], None,
                                OP.mult)
        if b_out_row is not None:
            nc.vector.tensor_add(crow_f[:], crow_f[:], b_out_row[:])
        crow_sb = const.tile([1, V], bf16)
        nc.vector.tensor_copy(crow_sb[:], crow_f[:])

        # decB = dec @ w_lin + crow  (crow broadcast over u via ones-row MM)
        psD = mpsum.tile([U, V], f32, tag="mps", name="psD")
        for k in range(KJ):
            for vh in range(2):
                nc.tensor.matmul(psD[:, vh * 512:(vh + 1) * 512],
                                 decT[:, k, :],
                                 wlin_sb[:, k, vh * 512:(vh + 1) * 512],
                                 start=(k == 0), stop=False)
        for vh in range(2):
            nc.tensor.matmul(psD[:, vh * 512:(vh + 1) * 512], ones_u[:],
                             crow_sb[:, vh * 512:(vh + 1) * 512],
                             start=False, stop=True)
        nc.vector.tensor_copy(decB_sb[:], psD[:])

        # encW block 0 (t 0:128)
        def encw_gemm(blk, eng):
            psE = mpsum.tile([P, V], f32, tag="mps", name=f"psE{blk}")
            for k in range(KJ):
                for vh in range(2):
                    nc.tensor.matmul(psE[:, vh * 512:(vh + 1) * 512],
                                     encT[:, k, blk * P:(blk + 1) * P],
                                     wlin_sb[:, k, vh * 512:(vh + 1) * 512],
                                     start=(k == 0), stop=(k == KJ - 1))
            eng.tensor_copy(encW_sb[:, blk, :], psE) if eng is nc.vector \
                else eng.copy(encW_sb[:, blk, :], psE)

        encw_gemm(0, nc.scalar)

        # L dec-part fill (SBUF->SBUF DMAs, partition shift +2), all bufs
        for jb in range(KJ):
            for bu in range(LBUF):
                nc.scalar.dma_start(L_all[2:2 + U, jb, bu, :], decB_sb[:])

        def asm_L(mm):
            # enc rows t0, t0+1 per row-block into L_all[0:2, jb, mm%LBUF]
            t0 = mm * TSUP
            blk, p0 = divmod(t0, P)
            for jb in range(KJ):
                nc.scalar.dma_start(
                    L_all[0:2, jb, mm % LBUF, :],
                    encW_sb[p0 + 2 * jb:p0 + 2 * jb + 2, blk, :])

        # ---- main loop ----
        out_r = out[:].rearrange("(mm j p) v -> mm p j v", j=KJ, p=P)

        joints = {}
        jqs = {}

        def build_joint_q(mm, h):
            # quarter h of supertile mm's joint + residual quant; quarter h
            # covers exactly the rows of matmul row-block jb=h
            t0 = mm * TSUP
            if h == 0:
                joint = jpool.tile([P, KJ, 512], bf16, tag="joint",
                                   name=f"jnt_{mm}")
                r2 = jpool.tile([P, KJ, 512], bf16, tag="r2", name=f"r2_{mm}")
                jq = jqpool.tile([P, KJ, 512], f8, tag="jq", name=f"jq_{mm}")
                joints[mm] = (joint, r2)
                jqs[mm] = jq
            else:
                joint, r2 = joints[mm]
                jq = jqs[mm]
            jv = joint.rearrange("p k (t u) -> p k t u", u=U)
            step = TSUP // 4
            tsl = slice(h * step, (h + 1) * step)
            enc_b = encT[:, :, t0 + h * step:t0 + (h + 1) * step, None] \
                .to_broadcast((P, KJ, step, U))
            dec_b = decT[:, :, None, :].to_broadcast((P, KJ, step, U))
            eng = nc.vector if h == 0 else nc.gpsimd
            eng.tensor_tensor(jv[:, :, tsl], dec_b, enc_b, OP.add)
            sl = slice(h * step * U, (h + 1) * step * U)
            # g' = |x| - 2b = (2*relu(x) - 2b) - x  (no abs ALU on trn2 DVE):
            # relu doubling on ACT, fused subtract chain on DVE, fp8e4 out
            nc.scalar.activation(out=r2[:, :, sl], in_=joint[:, :, sl],
                                 func=AF.Relu, scale=2.0)
            nc.vector.scalar_tensor_tensor(jq[:, :, sl], r2[:, :, sl],
                                           c2b_sb[:, 0:1], joint[:, :, sl],
                                           OP.subtract, OP.subtract)

        def build_joint(mm):
            for h in range(4):
                build_joint_q(mm, h)

        def supertile(mm):
            if mm + 1 < NMM:
                build_joint(mm + 1)
            if mm + 2 < NMM:
                asm_L(mm + 2)
            jq = jqs.pop(mm)
            joints.pop(mm)
            tail = mm >= NMM - 2
            for jb in range(KJ):
                pss = mpsum.tile([P, V], f32, tag="mps", name=f"ps_{mm}_{jb}")
                for kp in range(2):
                    for vh in range(2):
                        nc.tensor.matmul(
                            pss[:, vh * 512:(vh + 1) * 512],
                            jq[:, 2 * kp:2 * kp + 2, jb * P:(jb + 1) * P],
                            w8_sb[:, 2 * kp:2 * kp + 2,
                                  vh * 512:(vh + 1) * 512],
                            start=(kp == 0), stop=False, perf_mode=DR)
                for vh in range(2):
                    nc.tensor.matmul(pss[:, vh * 512:(vh + 1) * 512],
                                     s66_sb[:],
                                     L_all[:, jb, mm % LBUF,
                                           vh * 512:(vh + 1) * 512],
                                     start=False, stop=True)
                stage = opool.tile([P, V], bf16, tag="stage",
                                   name=f"st_{mm}_{jb}")
                if tail:
                    # split evictions/DMAs for a shorter serial tail
                    if jb % 2 == 0:
                        nc.scalar.copy(stage[:, :512], pss[:, :512])
                        nc.vector.tensor_copy(stage[:, 512:], pss[:, 512:])
                    else:
                        nc.vector.tensor_copy(stage[:, :512], pss[:, :512])
                        nc.scalar.copy(stage[:, 512:], pss[:, 512:])
                    nc.sync.dma_start(out_r[mm, :, jb, 0:512], stage[:, :512])
                    nc.sync.dma_start(out_r[mm, :, jb, 512:], stage[:, 512:])
                else:
                    if jb == 3:
                        nc.vector.tensor_copy(stage[:], pss[:])
                    else:
                        nc.scalar.copy(stage[:], pss[:])
                    nc.sync.dma_start(out_r[mm, :, jb], stage[:])

        build_joint(0)
        asm_L(0)
        asm_L(1)
        supertile(0)
        supertile(1)

        # deferred enc tb1 projection + LN (hides in main-loop slack)
        eps_mm1 = mpsum.tile([P, J], f32, tag="mps", name="emm_1")
        for k in range(E // P):
            nc.tensor.matmul(eps_mm1[:], xT_sb[:, k, P:2 * P], wenc_sb[:, k, :],
                             start=(k == 0), stop=(k == E // P - 1))
        encln1 = layer_norm_psum(eps_mm1, P, "e1", b_enc_sb, g_enc_sb, be_enc_sb)

        supertile(2)
        supertile(3)

        ept1 = mpsum.tile([P, KJ, P], bf16, tag="mps", name="ept_1")
        for jb in range(KJ):
            nc.tensor.transpose(ept1[:, jb, :], encln1[:, jb * P:(jb + 1) * P],
                                ident[:])
        nc.vector.tensor_copy(encT[:, :, P:2 * P], ept1[:])

        supertile(4)
        encw_gemm(1, nc.vector)

        for mm in range(5, NMM):
            supertile(mm)

    nc.compile()
    return nc


def _host_ln(y, g, be):
    mu = y.mean(-1, keepdims=True)
    yc = y - mu
    v = (yc * yc).mean(-1, keepdims=True)
    return yc / np.sqrt(v + EPS) * g + be


def _pack(a):
    # [K, N] -> [128, K//128, N] partition-major (SBUF layout)
    a = np.asarray(a)
    return np.ascontiguousarray(
        a.reshape(a.shape[0] // 128, 128, a.shape[1]).transpose(1, 0, 2))


def prepare(inputs):
    """-> (flags, named, in_maps) for the 8 cores."""
    import ml_dtypes

    bf = ml_dtypes.bfloat16
    f8 = ml_dtypes.float8_e4m3
    enc = np.asarray(inputs["encoder_out"], dtype=np.float32)
    dec = np.asarray(inputs["decoder_out"], dtype=np.float32)
    named = {}
    for k_src in ["b_enc", "g_enc", "be_enc", "b_dec", "g_dec", "be_dec",
                  "b_out"]:
        named[k_src] = np.ascontiguousarray(
            np.asarray(inputs[k_src], dtype=np.float32))
    for k_src, k_dst in [("W_enc", "w_enc"), ("W_dec", "w_dec")]:
        named[k_dst] = _pack(
            np.asarray(inputs[k_src], dtype=np.float32).astype(bf))
    W = np.asarray(inputs["W_out"], dtype=np.float32)
    named["w8"] = _pack((W * WS).astype(f8))
    named["w_lin"] = _pack((W * WS).astype(bf))

    # one-hot selector rows: 2 t-selectors + 64 u-selectors for a 128-row
    # block (2 t values x 64 u)
    r = np.arange(P)
    s = np.zeros((2 + U, P), np.float32)
    s[0] = r // U == 0
    s[1] = r // U == 1
    s[2 + r % U, r] = 1.0
    named["s66"] = np.ascontiguousarray(s.astype(bf))

    # 2b = E[|enc+dec|] from a T-subsample (any b is exact; the optimal b
    # minimizes the fp8 residual variance)
    We = np.asarray(inputs["W_enc"], np.float32)
    Wd = np.asarray(inputs["W_dec"], np.float32)
    e_s = _host_ln(enc[:, ::16] @ We + named["b_enc"], named["g_enc"],
                   named["be_enc"])
    d_s = _host_ln(dec @ Wd + named["b_dec"], named["g_dec"], named["be_dec"])
    c2b = np.abs(e_s[:, :, None, :] + d_s[:, None, :, :]).mean()
    named["c2b"] = np.array([c2b], dtype=np.float32)

    flags = (
        bool(np.any(named["b_enc"])), not np.all(named["g_enc"] == 1.0),
        bool(np.any(named["be_enc"])),
        bool(np.any(named["b_dec"])), not np.all(named["g_dec"] == 1.0),
        bool(np.any(named["be_dec"])),
        bool(np.any(named["b_out"])),
    )

    tpc = T // (NCORES // B)      # t-rows per core
    in_maps = []
    for c in range(NCORES):
        b = c // (NCORES // B)
        t0 = (c % (NCORES // B)) * tpc
        in_maps.append({
            "enc_xT": _pack(enc[b, t0:t0 + tpc].T.astype(bf)),
            "dec_xT": _pack(dec[b].T.astype(bf)),
            **named,
        })
    return flags, named, in_maps


def kernel(**inputs):
    from concourse.bass_utils import run_bass_kernel_spmd

    flags, _, in_maps = prepare(inputs)
    if flags not in _CACHE:
        _CACHE[flags] = _build(*flags)
    nc = _CACHE[flags]

    res = run_bass_kernel_spmd(nc, in_maps, core_ids=list(range(NCORES)))
    full = np.concatenate(
        [np.asarray(res.results[c]["out"]).astype(np.float32)
         for c in range(NCORES)], axis=0)
    full *= np.float32(OS)
    return full.reshape(B, T, U, V)
